# revision 8
# baseline (speedup 1.0000x reference)
"""Trainium2 Bass kernel for YOLO-style DetectionLoss.

Contract: kernel(**inputs) takes the FULL inputs (batch 512) and returns the
full output (5-tuple of f32 scalars), sharding batch-wise across 8 NeuronCores.

Per-core device program (64 images, 2048 GTs):
  - stream the predictions shard (12.5 MB) through SBUF in 10 contiguous DMAs
    with a tapered tail (the last chunks are small, so the post-stream ACT
    softplus cascade is short), accumulating sum(softplus(objectness)) over
    channels {0,5} of every cell (ACT exp -> ln(1+x) with accum_out)
  - compute GT cell row indices on-device from a small leading DMA (gt data
    first on the FIFO sync ring, consts second), then gather the 2048 GT
    cells with 16 indirect row-gather DMAs on gpsimd (HW semantics: one
    offset per partition + contiguous block), landing directly in the
    unified cells tile
  - decode boxes (sigmoid via exp+reciprocal; min(exp,1)); the responsible
    box is picked via cross-multiplied IoU comparison (no divide); sqrt and
    softplus terms are computed for BOTH boxes in ACT round 1 (straight off
    the gathered cells) so round 2 is only ln(class-sum) and never blocks
    the streaming softplus cascade
  - deduplicate cells holding >=1 GT (pairwise compare within each image via
    a partner-partition stream_shuffle) and subtract their softplus terms
    from the noobj sum
  - all 4+NCHUNK accumulators land in one (128, 4+NCHUNK) stats tile via
    accum_out; a single small DMA ships it to DRAM and the host sums over
    partitions and cores (no PE matmul reduce on the critical path).
"""
import sys

sys.path.insert(0, "/opt/trn_rl_repo")

import numpy as np

import concourse.bass as bass
import concourse.tile as tile
from concourse import bacc, mybir
from concourse.tile import add_dep_helper

S = 52
NBOX = 2
NCLS = 8
EPS = 1e-6
LAMBDA_COORD = 5.0
LAMBDA_NOOBJ = 0.5
BATCH = 512
N_GT = 32
NCORES = 8
NIMG = BATCH // NCORES          # 64 images per core
CELLS = S * S                   # 2704
ROWS = NIMG * CELLS             # 173056 rows of 18 floats per core
NG = NIMG * N_GT                # 2048 GTs per core
P = 128
JJ = NG // P                    # 16 GTs per partition
CH_ROWS = [186] * 6 + [90, 70, 46, 30]   # rows/partition per chunk, sum 1352
NCHUNK = len(CH_ROWS)
BIG = 1.0e7                     # invalid-GT row sentinel offset (exact in f32)

f32 = mybir.dt.float32
i32 = mybir.dt.int32
Act = mybir.ActivationFunctionType
Op = mybir.AluOpType
AxX = mybir.AxisListType.X

# meta_small layout (128, 112): row_base[16] | gtb[64] | gtl[16] | gtv[16]
#   row_base = (g // 32) * 2704, g = p*16+j
MS_W = 112
# meta_big layout (128, 268):
#   [0:8)     iota over classes 0..7
#   [8:264)   lower-strict-triangular mask tri[j*16+q] = 1.0 if q < j
#   [264]     parity = p % 2
#   [265]     EPS (1e-6)
#   [266:268) pad
MB_W = 268


def _build_meta_small_consts():
    c = np.zeros((P, 16), np.float32)
    p = np.arange(P)[:, None]
    j = np.arange(JJ)[None, :]
    g = p * JJ + j
    c[:, 0:16] = (g // N_GT) * CELLS
    return c


def _build_meta_big() -> np.ndarray:
    c = np.zeros((P, MB_W), np.float32)
    c[:, 0:8] = np.arange(NCLS)[None, :]
    tri = (np.arange(JJ)[None, :] < np.arange(JJ)[:, None]).astype(np.float32)
    c[:, 8:264] = tri.reshape(-1)[None, :]
    c[:, 264] = (np.arange(P) % 2).astype(np.float32)
    c[:, 265] = EPS
    return c


_ACT_PATCHED = False


def _force_single_act_table():
    """Make the act-table-load pass place every activation in
    natural_log_exp_and_others (covers Exp+Ln), so the kernel pays one
    ACT_TABLE_LOAD instead of thrashing between per-function sets."""
    global _ACT_PATCHED
    if _ACT_PATCHED:
        return
    from concourse import hw_specs

    orig = hw_specs.get_activation_tables

    def patched(arch):
        t = orig(arch)
        keep = "natural_log_exp_and_others"
        if keep not in t:
            return t
        return {k: (v if k == keep else set()) for k, v in t.items()}

    hw_specs.get_activation_tables = patched
    bacc.get_activation_tables = patched
    _ACT_PATCHED = True


def build_program(for_sim: bool = False, debug: bool = False) -> bass.Bass:
    _force_single_act_table()
    nc = bacc.Bacc(None, target_bir_lowering=False,
                   dynamic_dma_scratch_size=65536)

    pred = nc.dram_tensor("pred", [ROWS, 18], f32, kind="ExternalInput")
    ms_d = nc.dram_tensor("ms", [P, MS_W], f32, kind="ExternalInput")
    mb_d = nc.dram_tensor("mb", [P, MB_W], f32, kind="ExternalInput")
    out_d = nc.dram_tensor("out", [P, 4 + NCHUNK], f32, kind="ExternalOutput")

    shuffle_mask = []
    for i in range(0, 32, 2):
        shuffle_mask += [i + 1, i]

    with tile.TileContext(nc) as tc:
        with (
            tc.tile_pool(name="main", bufs=1) as mp,
            tc.tile_pool(name="stream", bufs=1) as sp,
        ):
            # one stats tile: cols 0..3 = coord/obj/cls/corr (DVE accums),
            # cols 4.. = per-chunk noobj softplus sums (ACT accums)
            stats = mp.tile([P, 4 + NCHUNK], f32)

            ch_base = [0]
            for r in CH_ROWS:
                ch_base.append(ch_base[-1] + P * r)

            ln_insts = {}

            def stream_dma(c):
                r = CH_ROWS[c]
                st = sp.tile([P, r * 18], f32, tag=f"st{c}")
                srcv = pred[ch_base[c]:ch_base[c + 1], :].rearrange(
                    "(p f) d -> p (f d)", p=P)
                nc.sync.dma_start(out=st[:], in_=srcv)
                return st

            def stream_act(c, st):
                r = CH_ROWS[c]
                st3 = st[:].rearrange("p (f d) -> p f d", d=18)
                e = sp.tile([P, r * 2], f32, tag=f"spe{c}")
                e3 = e[:].rearrange("p (f d) -> p f d", d=2)
                nc.scalar.activation(out=e3, in_=st3[:, :, 0:10:5], func=Act.Exp)
                sl = sp.tile([P, r * 2], f32, tag=f"spl{c}")
                ln_insts[c] = nc.scalar.activation(
                    out=sl[:], in_=e[:], func=Act.Ln, bias=1.0,
                    accum_out=stats[:, 4 + c:5 + c])

            # ---- small gt-data DMA first on the FIFO sync ring (lands before
            #      the bulk stream floods the DMA engines), then consts, then
            #      the bulk chunks
            ms = mp.tile([P, MS_W], f32)
            nc.sync.dma_start(out=ms[:], in_=ms_d[:])
            mb = mp.tile([P, MB_W], f32)
            nc.sync.dma_start(out=mb[:], in_=mb_d[:])
            row_base = ms[:, 0:16]
            gtb = ms[:, 16:16 + 4 * JJ]
            gtl = ms[:, 16 + 4 * JJ:16 + 5 * JJ]
            gtv = ms[:, 16 + 5 * JJ:16 + 6 * JJ]
            gtb3 = gtb.rearrange("p (j c) -> p j c", c=4)
            iota8 = mb[:, 0:8]
            tri = mb[:, 8:264]
            parity = mb[:, 264:265]
            epsc = mb[:, 265:266]

            st_tiles = [stream_dma(c) for c in range(NCHUNK)]

            # ---- GT cell indices: gj from cx, gi from cy (layout (j, c):
            #      c=0 -> gj, c=1 -> gi)
            t52 = mp.tile([P, 2 * JJ], f32)
            t52v = t52[:].rearrange("p (j c) -> p j c", c=2)
            nc.vector.tensor_scalar(t52v, gtb3[:, :, 0:2], 52.0, None, Op.mult)
            # floor(x): int cast rounds-to-nearest on HW (truncates in sim);
            # r - (r > x) is exact floor under either behavior for x >= 0
            gjii = mp.tile([P, 2 * JJ], i32)
            nc.vector.tensor_copy(out=gjii[:], in_=t52[:])
            gjif = mp.tile([P, 2 * JJ], f32)
            nc.vector.tensor_copy(out=gjif[:], in_=gjii[:])
            gadj = mp.tile([P, 2 * JJ], f32)
            nc.vector.tensor_tensor(gadj[:], gjif[:], t52[:], op=Op.is_gt)
            gjfl = mp.tile([P, 2 * JJ], f32)
            nc.vector.tensor_tensor(gjfl[:], gjif[:], gadj[:], op=Op.subtract)
            gji = mp.tile([P, 2 * JJ], f32)
            nc.vector.tensor_scalar(gji[:], gjfl[:], 51.0, 0.0, Op.min, Op.max)
            gji3 = gji[:].rearrange("p (j c) -> p j c", c=2)

            # row = row_base + gi*52 + gj
            rowa = mp.tile([P, JJ], f32)
            nc.vector.scalar_tensor_tensor(
                out=rowa[:], in0=gji3[:, :, 1], scalar=52.0, in1=gji3[:, :, 0],
                op0=Op.mult, op1=Op.add)
            rowf = mp.tile([P, JJ], f32)
            nc.vector.tensor_tensor(
                out=rowf[:], in0=rowa[:], in1=row_base, op=Op.add)
            rowi = mp.tile([P, JJ], i32)
            nc.vector.tensor_copy(out=rowi[:], in_=rowf[:])

            # ---- 16 row-gathers (one offset per partition each), landing
            #      directly in the unified cells tile's column slices
            cells = mp.tile([P, JJ * 18], f32)
            cells3 = cells[:].rearrange("p (j c) -> p j c", c=18)
            for j in range(JJ):
                nc.gpsimd.indirect_dma_start(
                    out=cells[:, j * 18:(j + 1) * 18],
                    out_offset=None,
                    in_=pred[:],
                    in_offset=bass.IndirectOffsetOnAxis(
                        ap=rowi[:, j:j + 1], axis=0),
                )

            # ---- dedup (needs only rowf/gtv): cells holding >=1 valid GT,
            #      counted once per image via partner-partition shuffle
            rowm_a = mp.tile([P, JJ], f32)
            nc.vector.scalar_tensor_tensor(
                out=rowm_a[:], in0=gtv, scalar=BIG, in1=rowf[:],
                op0=Op.mult, op1=Op.add)
            rowm = mp.tile([P, JJ], f32)
            nc.vector.tensor_scalar(rowm[:], rowm_a[:], -BIG, None, Op.add)
            rowp = mp.tile([P, JJ], f32)
            nc.vector.stream_shuffle(out=rowp[:], in_=rowm[:], mask=shuffle_mask)
            rmj = rowm[:].unsqueeze(2).to_broadcast([P, JJ, JJ])
            rmq = rowm[:].unsqueeze(1).to_broadcast([P, JJ, JJ])
            rpq = rowp[:].unsqueeze(1).to_broadcast([P, JJ, JJ])
            cmps = mp.tile([P, JJ * JJ], f32)
            cmps3 = cmps[:].rearrange("p (j q) -> p j q", q=JJ)
            nc.vector.tensor_tensor(cmps3, rmj, rmq, op=Op.is_equal)
            prods = mp.tile([P, JJ * JJ], f32)
            nc.vector.tensor_tensor(prods[:], cmps[:], tri, op=Op.mult)
            cnts = mp.tile([P, JJ], f32)
            nc.vector.tensor_reduce(
                cnts[:], prods[:].rearrange("p (j q) -> p j q", q=JJ),
                axis=AxX, op=Op.add)
            cmpp = mp.tile([P, JJ * JJ], f32)
            cmpp3 = cmpp[:].rearrange("p (j q) -> p j q", q=JJ)
            nc.vector.tensor_tensor(cmpp3, rmj, rpq, op=Op.is_equal)
            cntp = mp.tile([P, JJ], f32)
            nc.vector.tensor_reduce(
                cntp[:], cmpp[:].rearrange("p (j q) -> p j q", q=JJ),
                axis=AxX, op=Op.add)
            dup = mp.tile([P, JJ], f32)
            nc.vector.scalar_tensor_tensor(
                out=dup[:], in0=cntp[:], scalar=parity, in1=cnts[:],
                op0=Op.mult, op1=Op.add)
            wd = mp.tile([P, JJ], f32)
            nc.vector.tensor_scalar(wd[:], dup[:], 0.0, None, Op.is_equal)
            wv = mp.tile([P, JJ], f32)
            nc.vector.tensor_tensor(wv[:], wd[:], gtv, op=Op.mult)

            # gt-box geometry (needs only gt data): corners, area(+EPS), sqrt
            wh2g = mp.tile([P, 2 * JJ], f32)
            wh2g3 = wh2g[:].rearrange("p (j c) -> p j c", c=2)
            nc.vector.tensor_scalar(wh2g3, gtb3[:, :, 2:4], 0.5, None, Op.mult)
            g1 = mp.tile([P, 2 * JJ], f32)
            g13 = g1[:].rearrange("p (j c) -> p j c", c=2)
            nc.vector.tensor_tensor(g13, gtb3[:, :, 0:2], wh2g3, op=Op.subtract)
            g2 = mp.tile([P, 2 * JJ], f32)
            g23 = g2[:].rearrange("p (j c) -> p j c", c=2)
            nc.vector.tensor_tensor(g23, gtb3[:, :, 0:2], wh2g3, op=Op.add)
            g1b = g13.unsqueeze(2).to_broadcast([P, JJ, 2, 2])
            g2b = g23.unsqueeze(2).to_broadcast([P, JJ, 2, 2])
            a2 = mp.tile([P, JJ], f32)
            nc.vector.tensor_tensor(
                a2[:], gtb3[:, :, 2], gtb3[:, :, 3], op=Op.mult)
            a2e = mp.tile([P, JJ], f32)   # gt area + EPS folded in early
            nc.vector.tensor_scalar(a2e[:], a2[:], EPS, None, Op.add)
            a2b = a2e[:].unsqueeze(2).to_broadcast([P, JJ, 2])
            # sqrt(x + EPS) = exp(0.5 * ln(x + EPS)); single ACT table set
            lng = mp.tile([P, 2 * JJ], f32)
            lng3 = lng[:].rearrange("p (j c) -> p j c", c=2)
            nc.scalar.activation(
                out=lng3, in_=gtb3[:, :, 2:4], func=Act.Ln, bias=epsc)
            syg = mp.tile([P, 2 * JJ], f32)
            nc.scalar.activation(out=syg[:], in_=lng[:], func=Act.Exp, scale=0.5)

            # ---- streaming softplus chunks 0..5 on ACT
            for c in range(6):
                stream_act(c, st_tiles[c])

            # ---- ACT round 1: everything computable straight off cells.
            #      sqrt and softplus terms are computed for BOTH boxes here
            #      so ACT round 2 shrinks to a single ln (class sums).
            txy_in = cells3[:, :, 1:11].rearrange(
                "p j (k f) -> p j k f", k=2)[:, :, :, 0:2]
            exy = mp.tile([P, 4 * JJ], f32)
            exy4 = exy[:].rearrange("p (j k c) -> p j k c", k=2, c=2)
            exy_inst = nc.scalar.activation(
                out=exy4, in_=txy_in, func=Act.Exp, scale=-1.0)
            add_dep_helper(exy_inst.ins, ln_insts[3].ins, False,
                           "order per-GT ACT round 1 after chunk3 softplus")
            twh_in = cells3[:, :, 3:13].rearrange(
                "p j (k f) -> p j k f", k=2)[:, :, :, 0:2]
            ewh = mp.tile([P, 4 * JJ], f32)
            ewh4 = ewh[:].rearrange("p (j k c) -> p j k c", k=2, c=2)
            ewh_inst = nc.scalar.activation(out=ewh4, in_=twh_in, func=Act.Exp)
            add_dep_helper(ewh_inst.ins, ln_insts[3].ins, False, "act order")
            # class: exp of raw logits (|x| small, no max-shift needed)
            ecls = mp.tile([P, NCLS * JJ], f32)
            ecls3 = ecls[:].rearrange("p (j c) -> p j c", c=NCLS)
            ecls_inst = nc.scalar.activation(
                out=ecls3, in_=cells3[:, :, 10:18], func=Act.Exp)
            add_dep_helper(ecls_inst.ins, ln_insts[3].ins, False, "act order")
            # noobj correction softplus(+t) of the two objectness logits
            ec = mp.tile([P, 2 * JJ], f32)
            ec3 = ec[:].rearrange("p (j c) -> p j c", c=2)
            ec_inst = nc.scalar.activation(
                out=ec3, in_=cells3[:, :, 0:10:5], func=Act.Exp)
            add_dep_helper(ec_inst.ins, ln_insts[3].ins, False, "act order")
            scn = mp.tile([P, 2 * JJ], f32)
            scn_inst = nc.scalar.activation(
                out=scn[:], in_=ec[:], func=Act.Ln, bias=1.0)
            add_dep_helper(scn_inst.ins, ln_insts[3].ins, False, "act order")
            # obj BCE softplus(-t) for BOTH boxes
            eob = mp.tile([P, 2 * JJ], f32)
            eob3 = eob[:].rearrange("p (j c) -> p j c", c=2)
            eob_inst = nc.scalar.activation(
                out=eob3, in_=cells3[:, :, 0:10:5], func=Act.Exp, scale=-1.0)
            add_dep_helper(eob_inst.ins, ln_insts[3].ins, False, "act order")
            sob = mp.tile([P, 2 * JJ], f32)
            sob_inst = nc.scalar.activation(
                out=sob[:], in_=eob[:], func=Act.Ln, bias=1.0)
            add_dep_helper(sob_inst.ins, ln_insts[3].ins, False, "act order")
            sob3 = sob[:].rearrange("p (j k) -> p j k", k=2)

            # ---- DVE: clamp pw/ph for both boxes (feeds ACT sqrt round 1b)
            pwh = mp.tile([P, 4 * JJ], f32)
            nc.vector.tensor_scalar(pwh[:], ewh[:], 1.0, None, Op.min)
            pwh4 = pwh[:].rearrange("p (j k c) -> p j k c", k=2, c=2)

            # ---- ACT round 1b: sqrt(pwh + EPS) for BOTH boxes
            lnp = mp.tile([P, 4 * JJ], f32)
            lnp_inst = nc.scalar.activation(
                out=lnp[:], in_=pwh[:], func=Act.Ln, bias=epsc)
            add_dep_helper(lnp_inst.ins, ln_insts[3].ins, False, "act order")
            syp = mp.tile([P, 4 * JJ], f32)
            syp_inst = nc.scalar.activation(
                out=syp[:], in_=lnp[:], func=Act.Exp, scale=0.5)
            add_dep_helper(syp_inst.ins, ln_insts[3].ins, False, "act order")
            syp4 = syp[:].rearrange("p (j k c) -> p j k c", k=2, c=2)

            # ---- DVE mid-chain: box decode, IoU, responsible box
            # class sums first: they feed the only ACT round-2 instruction
            sm = mp.tile([P, JJ], f32)
            nc.vector.tensor_reduce(
                sm[:], ecls[:].rearrange("p (j c) -> p j c", c=NCLS),
                axis=AxX, op=Op.add)
            den = mp.tile([P, 4 * JJ], f32)
            nc.vector.tensor_scalar(den[:], exy[:], 1.0, None, Op.add)
            sgm = mp.tile([P, 4 * JJ], f32)
            nc.vector.reciprocal(sgm[:], den[:])
            sgm4 = sgm[:].rearrange("p (j k c) -> p j k c", k=2, c=2)
            # px = (sigmoid + gj) * fl(1/52)
            gjib = gji[:].rearrange(
                "p (j c) -> p j c", c=2).unsqueeze(2).to_broadcast([P, JJ, 2, 2])
            sgp = mp.tile([P, 4 * JJ], f32)
            sgp4 = sgp[:].rearrange("p (j k c) -> p j k c", k=2, c=2)
            nc.vector.tensor_tensor(sgp4, sgm4, gjib, op=Op.add)
            pxy = mp.tile([P, 4 * JJ], f32)
            pxy4 = pxy[:].rearrange("p (j k c) -> p j k c", k=2, c=2)
            nc.vector.tensor_scalar(pxy[:], sgp[:], 1.0 / S, None, Op.mult)
            # corners: p1 = pxy - pwh*0.5, p2 = pxy + pwh*0.5 (fused)
            p1 = mp.tile([P, 4 * JJ], f32)
            nc.vector.scalar_tensor_tensor(
                out=p1[:], in0=pwh[:], scalar=-0.5, in1=pxy[:],
                op0=Op.mult, op1=Op.add)
            p14 = p1[:].rearrange("p (j k c) -> p j k c", k=2, c=2)
            p2 = mp.tile([P, 4 * JJ], f32)
            nc.vector.scalar_tensor_tensor(
                out=p2[:], in0=pwh[:], scalar=0.5, in1=pxy[:],
                op0=Op.mult, op1=Op.add)
            p24 = p2[:].rearrange("p (j k c) -> p j k c", k=2, c=2)
            lo = mp.tile([P, 4 * JJ], f32)
            lo4 = lo[:].rearrange("p (j k c) -> p j k c", k=2, c=2)
            nc.vector.tensor_tensor(lo4, p14, g1b, op=Op.max)
            hi = mp.tile([P, 4 * JJ], f32)
            hi4 = hi[:].rearrange("p (j k c) -> p j k c", k=2, c=2)
            nc.vector.tensor_tensor(hi4, p24, g2b, op=Op.min)
            iwr = mp.tile([P, 4 * JJ], f32)
            nc.vector.tensor_tensor(iwr[:], hi[:], lo[:], op=Op.subtract)
            iwh = mp.tile([P, 4 * JJ], f32)
            nc.vector.tensor_scalar(iwh[:], iwr[:], 0.0, None, Op.max)
            iwh4 = iwh[:].rearrange("p (j k c) -> p j k c", k=2, c=2)
            inter = mp.tile([P, 2 * JJ], f32)
            inter3 = inter[:].rearrange("p (j k) -> p j k", k=2)
            nc.vector.tensor_tensor(
                inter3, iwh4[:, :, :, 0], iwh4[:, :, :, 1], op=Op.mult)
            # pred areas from clamped wh; union = a1 + (a2 + EPS) - inter
            a1 = mp.tile([P, 2 * JJ], f32)
            a13 = a1[:].rearrange("p (j k) -> p j k", k=2)
            nc.vector.tensor_tensor(
                a13, pwh4[:, :, :, 0], pwh4[:, :, :, 1], op=Op.mult)
            u1 = mp.tile([P, 2 * JJ], f32)
            u13 = u1[:].rearrange("p (j k) -> p j k", k=2)
            nc.vector.tensor_tensor(u13, a13, a2b, op=Op.add)
            un = mp.tile([P, 2 * JJ], f32)
            un3 = un[:].rearrange("p (j k) -> p j k", k=2)
            nc.vector.tensor_tensor(un[:], u1[:], inter[:], op=Op.subtract)
            # responsible box: iou1 > iou0  <=>  inter1*un0 > inter0*un1
            m1 = mp.tile([P, JJ], f32)
            nc.vector.tensor_tensor(
                m1[:], inter3[:, :, 1], un3[:, :, 0], op=Op.mult)
            m0 = mp.tile([P, JJ], f32)
            nc.vector.tensor_tensor(
                m0[:], inter3[:, :, 0], un3[:, :, 1], op=Op.mult)
            sel = mp.tile([P, JJ], i32)   # int mask for copy_predicated
            nc.vector.tensor_tensor(sel[:], m1[:], m0[:], op=Op.is_gt)
            selb = sel[:].unsqueeze(2).to_broadcast([P, JJ, 2])

            # picks via copy + predicated overwrite (2 ops each)
            bxy = mp.tile([P, 2 * JJ], f32)
            bxy3 = bxy[:].rearrange("p (j c) -> p j c", c=2)
            nc.vector.tensor_copy(out=bxy3, in_=pxy4[:, :, 0, :])
            nc.vector.copy_predicated(out=bxy3, mask=selb, data=pxy4[:, :, 1, :])
            sow = mp.tile([P, JJ], f32)
            nc.vector.tensor_copy(out=sow[:], in_=sob3[:, :, 0])
            nc.vector.copy_predicated(out=sow[:], mask=sel[:], data=sob3[:, :, 1])
            syw = mp.tile([P, 2 * JJ], f32)
            syw3 = syw[:].rearrange("p (j c) -> p j c", c=2)
            nc.vector.tensor_copy(out=syw3, in_=syp4[:, :, 0, :])
            nc.vector.copy_predicated(out=syw3, mask=selb, data=syp4[:, :, 1, :])

            # coord xy part
            dxy = mp.tile([P, 2 * JJ], f32)
            dxy3 = dxy[:].rearrange("p (j c) -> p j c", c=2)
            nc.vector.tensor_tensor(dxy3, bxy3, gtb3[:, :, 0:2], op=Op.subtract)
            dxy2 = mp.tile([P, 2 * JJ], f32)
            nc.vector.tensor_tensor(dxy2[:], dxy[:], dxy[:], op=Op.mult)
            cdxy = mp.tile([P, JJ], f32)
            nc.vector.tensor_reduce(
                cdxy[:], dxy2[:].rearrange("p (j c) -> p j c", c=2),
                axis=AxX, op=Op.add)
            # coord wh part (sqrt diffs of the responsible box)
            dwh = mp.tile([P, 2 * JJ], f32)
            nc.vector.tensor_tensor(dwh[:], syw[:], syg[:], op=Op.subtract)
            dwh2 = mp.tile([P, 2 * JJ], f32)
            nc.vector.tensor_tensor(dwh2[:], dwh[:], dwh[:], op=Op.mult)
            cdwh = mp.tile([P, JJ], f32)
            nc.vector.tensor_reduce(
                cdwh[:], dwh2[:].rearrange("p (j c) -> p j c", c=2),
                axis=AxX, op=Op.add)
            coordt = mp.tile([P, JJ], f32)
            nc.vector.tensor_tensor(coordt[:], cdxy[:], cdwh[:], op=Op.add)
            coordv = mp.tile([P, JJ], f32)
            nc.vector.scalar_tensor_tensor(
                out=coordv[:], in0=coordt[:], scalar=1.0, in1=gtv,
                op0=Op.mult, op1=Op.mult, accum_out=stats[:, 0:1])
            # obj loss from the picked softplus
            objv = mp.tile([P, JJ], f32)
            nc.vector.scalar_tensor_tensor(
                out=objv[:], in0=sow[:], scalar=1.0, in1=gtv,
                op0=Op.mult, op1=Op.mult, accum_out=stats[:, 1:2])
            # noobj correction pair-sum and weighting
            spc = mp.tile([P, JJ], f32)
            nc.vector.tensor_reduce(
                spc[:], scn[:].rearrange("p (j c) -> p j c", c=2),
                axis=AxX, op=Op.add)
            corrv = mp.tile([P, JJ], f32)
            nc.vector.scalar_tensor_tensor(
                out=corrv[:], in0=spc[:], scalar=1.0, in1=wv[:],
                op0=Op.mult, op1=Op.mult, accum_out=stats[:, 3:4])
            # class: picked logit
            oh = mp.tile([P, NCLS * JJ], f32)
            oh3 = oh[:].rearrange("p (j c) -> p j c", c=NCLS)
            gtlb = gtl.unsqueeze(2).to_broadcast([P, JJ, NCLS])
            iotb = iota8.unsqueeze(1).to_broadcast([P, JJ, NCLS])
            nc.vector.tensor_tensor(oh3, gtlb, iotb, op=Op.is_equal)
            pick = mp.tile([P, NCLS * JJ], f32)
            pick3 = pick[:].rearrange("p (j c) -> p j c", c=NCLS)
            nc.vector.tensor_tensor(pick3, oh3, cells3[:, :, 10:18], op=Op.mult)
            lab = mp.tile([P, JJ], f32)
            nc.vector.tensor_reduce(
                lab[:], pick[:].rearrange("p (j c) -> p j c", c=NCLS),
                axis=AxX, op=Op.add)

            # ---- ACT round 2: only ln(class sums); placed before the tail
            #      chunk pairs so those run last and back-to-back
            ls = mp.tile([P, JJ], f32)
            ls_inst = nc.scalar.activation(out=ls[:], in_=sm[:], func=Act.Ln)
            add_dep_helper(ls_inst.ins, ln_insts[3].ins, False,
                           "class-sum ln after chunk3 softplus")

            # ---- streaming chunks 6..end (tapered tail)
            for c in range(6, NCHUNK):
                stream_act(c, st_tiles[c])

            # ---- DVE tail: class loss accumulation
            nll = mp.tile([P, JJ], f32)
            nc.vector.tensor_tensor(nll[:], ls[:], lab[:], op=Op.subtract)
            nllv = mp.tile([P, JJ], f32)
            nc.vector.scalar_tensor_tensor(
                out=nllv[:], in0=nll[:], scalar=1.0, in1=gtv,
                op0=Op.mult, op1=Op.mult, accum_out=stats[:, 2:3])

            # ---- ship the raw per-partition stats; host reduces.
            #      Issued from the ACT engine: the final chunk's accumulator
            #      read is the last producer, so this skips a cross-engine
            #      semaphore hop to Sync.
            nc.scalar.dma_start(out=out_d[:], in_=stats[:])

    nc.compile()
    return nc


_MS_CONSTS = _build_meta_small_consts()
_MB = _build_meta_big()
_NC_CACHE = {}


def _get_program(for_sim: bool = False) -> bass.Bass:
    key = bool(for_sim)
    if key not in _NC_CACHE:
        _NC_CACHE[key] = build_program(for_sim)
    return _NC_CACHE[key]


def make_in_maps(predictions, gt_boxes, gt_labels, gt_valid):
    predictions = np.ascontiguousarray(np.asarray(predictions), np.float32)
    gtb = np.ascontiguousarray(np.asarray(gt_boxes), np.float32)
    gtl = np.asarray(gt_labels).astype(np.float32)
    gtv = np.asarray(gt_valid).astype(np.float32)
    in_maps = []
    for c in range(NCORES):
        sl = slice(c * NIMG, (c + 1) * NIMG)
        ms = np.concatenate([
            _MS_CONSTS,
            gtb[sl].reshape(NG, 4).reshape(P, JJ * 4),
            gtl[sl].reshape(NG).reshape(P, JJ),
            gtv[sl].reshape(NG).reshape(P, JJ),
        ], axis=1)
        in_maps.append({
            "pred": predictions[sl].reshape(ROWS, 18),
            "ms": np.ascontiguousarray(ms),
            "mb": _MB,
        })
    return in_maps


def combine_outputs(outs):
    """outs: list of (P, 4+NCHUNK) per-core partials -> 5-tuple of scalars."""
    t = np.stack([np.asarray(o).reshape(P, 4 + NCHUNK) for o in outs])
    s = t.astype(np.float64).sum(axis=(0, 1))
    coord, obj, cls, corr = s[0], s[1], s[2], s[3]
    noobj = s[4:4 + NCHUNK].sum() - corr
    total = (LAMBDA_COORD * coord + obj + LAMBDA_NOOBJ * noobj + cls) / BATCH
    return (np.float32(total), np.float32(coord / BATCH),
            np.float32(obj / BATCH), np.float32(noobj / BATCH),
            np.float32(cls / BATCH))


def kernel(predictions, gt_boxes, gt_labels, gt_valid):
    from concourse.bass_utils import run_bass_kernel_spmd

    nc = _get_program(for_sim=False)
    in_maps = make_in_maps(predictions, gt_boxes, gt_labels, gt_valid)
    try:
        res = run_bass_kernel_spmd(nc, in_maps, list(range(NCORES))).results
    except Exception:
        # transient NRT_EXEC_UNIT_UNRECOVERABLE has been observed right
        # after an earlier crashed run; one retry clears it
        res = run_bass_kernel_spmd(nc, in_maps, list(range(NCORES))).results
    return combine_outputs([r["out"] for r in res])
